# revision 16
# baseline (speedup 1.0000x reference)
"""Trainium2 Bass kernel for nn_ContinuousPool.

Computes, for x:(32,96,128,128) f32 and pool_strength:(1,96,1,1) f32:
    cur = x
    repeat 10: cur = cur + s * (maxpool3x3_same(cur) - cur)
    out = avgpool2x2(cur)            -> (32,96,64,64)

Strategy (v6):
  - Pure data parallel over 8 cores: 4 batches/core -> 384 images/core,
    processed as 3 chunks of 128 images (one image per SBUF partition).
  - State in fp16 (end-to-end error ~1.8e-3 vs the 2e-2 tolerance).
    DVE tensor_tensor runs at 2 elem/cycle on flat contiguous APs.
  - Measured on HW: a back-to-back DEPENDENT DVE op costs ~10.7us per
    16K elements vs ~7.8us independent (write-visibility latency). So
    every step pass is split into top/bottom image halves and the two
    halves' ops are interleaved - consecutive DVE instructions are
    independent and the chain latency is hidden.
  - u is stored FLAT [128, 16384] (no column pads): the row max runs
    over the flat array and two tiny fixup ops rewrite the image
    columns where the flat shift wraps across row boundaries.
    r [130,128] has NEG pad rows for the column max. Blend
    u' = u + c*max3x3(u), c = s/(1-s), is tensor_scalar_mul (into the
    idle second u buffer) + in-place tensor_add; (1-s)^10/4 folds into
    the avgpool scale.
  - All compute on DVE (Pool-engine tensor ops don't compile through
    neuronxcc; Act-engine fp16 output is pathologically slow). Input
    f32->fp16 conversion is a DVE tensor_copy from two staged
    half-chunk buffers, DMA-preloaded during the previous chunk's
    steps. The f32 avgpool output reuses the dead r tile via bitcast.
"""

import sys

import numpy as np

if "/opt/trn_rl_repo" not in sys.path:
    sys.path.insert(0, "/opt/trn_rl_repo")

B, C, H, W = 32, 96, 128, 128
T = 10
N_CORES = 8
B_PER_CORE = B // N_CORES          # 4
IMGS = B_PER_CORE * C              # 384 images per core
CHUNK = 128
NCHUNK = IMGS // CHUNK             # 3
HW_ = H * W                        # 16384
HH = HW_ // 2                      # 8192 (one half, 64 image rows)
NEG = -60000.0

_CACHE = {}


def _build_program(reps=None, bodies=1):
    import concourse.bacc as bacc
    import concourse.mybir as mybir
    from concourse import tile

    f16 = mybir.dt.float16
    f32 = mybir.dt.float32

    nc = bacc.Bacc("TRN2", target_bir_lowering=False, debug=False,
                   num_devices=N_CORES)

    x_d = nc.dram_tensor("x", [IMGS, HW_], f32, kind="ExternalInput")
    c_d = nc.dram_tensor("cvec", [IMGS, 1], f32, kind="ExternalInput")
    f_d = nc.dram_tensor("fvec", [IMGS, 1], f32, kind="ExternalInput")
    y_d = nc.dram_tensor("y", [IMGS, HW_ // 4], f32, kind="ExternalOutput")

    with tile.TileContext(nc, num_cores=N_CORES) as tc:
        with tc.tile_pool(name="main", bufs=1) as pool:
            u_ts = [pool.tile([128, HW_], f16, name=f"u{i}", tag=f"u{i}")
                    for i in (0, 1)]
            r_t = pool.tile([128, 130 * W], f16, tag="r")
            v_t = pool.tile([128, HW_], f16, tag="v")
            st_ts = [pool.tile([128, HH], f32, name=f"st{i}",
                               tag=f"st{i}") for i in (0, 1)]
            cs_t = pool.tile([128, 2 * NCHUNK], f32, tag="cs")

            # one-time init: r's NEG pads (and the 2 elements the flat
            # row pass never writes); u scratch for uninit-read checks
            nc.gpsimd.memset(r_t[:, :], NEG)
            nc.gpsimd.memset(u_ts[1][:, :], 0.0)
            for k in range(NCHUNK):
                rows = slice(k * CHUNK, (k + 1) * CHUNK)
                nc.sync.dma_start(cs_t[:, 2 * k:2 * k + 1], c_d[rows, :])
                nc.sync.dma_start(cs_t[:, 2 * k + 1:2 * k + 2], f_d[rows, :])

            def t3(t, h=H):
                return t[:, 0:h * W].rearrange("p (h w) -> p h w", h=h, w=W)

            def dma_in(k, half):
                rows = slice(k * CHUNK, (k + 1) * CHUNK)
                nc.sync.dma_start(st_ts[half][:, :],
                                  x_d[rows, half * HH:(half + 1) * HH])

            def convert(u_t, half):
                nc.vector.tensor_copy(u_t[:, half * HH:(half + 1) * HH],
                                      st_ts[half][:, :])

            def step(u_t, w_t, k):
                """One evolution step; halves interleaved so consecutive
                DVE ops are independent. w_t is the idle other-u buffer
                used as the scaled-update scratch."""
                uv, rv = t3(u_t), t3(r_t, 130)
                # 1. flat row-neighbor max -> r rows 1..128 (off-by-W)
                nc.vector.tensor_max(r_t[:, W + 1:W + HH],
                                     u_t[:, 0:HH - 1],
                                     u_t[:, 2:HH + 1])
                nc.vector.tensor_max(r_t[:, W + HH:W + HW_ - 1],
                                     u_t[:, HH - 1:HW_ - 2],
                                     u_t[:, HH + 1:HW_])
                # 2. merge center (in-place on r)
                nc.vector.tensor_max(r_t[:, W:W + HH], r_t[:, W:W + HH],
                                     u_t[:, 0:HH])
                nc.vector.tensor_max(r_t[:, W + HH:W + HW_],
                                     r_t[:, W + HH:W + HW_],
                                     u_t[:, HH:HW_])
                # fix image cols 0 and 127 (flat shift wrapped rows);
                # split at r row 66 so pass 3's T half (reads r rows
                # 0..65) depends on the 2-back op, not the adjacent one
                nc.vector.tensor_max(rv[:, 1:67, 0:1], uv[:, 0:66, 0:1],
                                     uv[:, 0:66, 1:2])
                nc.vector.tensor_max(rv[:, 1:67, 127:128],
                                     uv[:, 0:66, 126:127],
                                     uv[:, 0:66, 127:128])
                nc.vector.tensor_max(rv[:, 67:129, 0:1], uv[:, 66:128, 0:1],
                                     uv[:, 66:128, 1:2])
                nc.vector.tensor_max(rv[:, 67:129, 127:128],
                                     uv[:, 66:128, 126:127],
                                     uv[:, 66:128, 127:128])
                # 3. column neighbor max -> v
                nc.vector.tensor_max(v_t[:, 0:HH], r_t[:, 0:HH],
                                     r_t[:, 2 * W:2 * W + HH])
                nc.vector.tensor_max(v_t[:, HH:HW_], r_t[:, HH:HH + HH],
                                     r_t[:, 2 * W + HH:2 * W + HW_])
                # 4. merge center row (in-place on v) = max3x3(u)
                nc.vector.tensor_max(v_t[:, 0:HH], v_t[:, 0:HH],
                                     r_t[:, W:W + HH])
                nc.vector.tensor_max(v_t[:, HH:HW_], v_t[:, HH:HW_],
                                     r_t[:, W + HH:W + HW_])
                # 5. w = c*v (tensor_scalar, ~8 elem/cyc, distinct dst)
                cs = cs_t[:, 2 * k:2 * k + 1]
                nc.vector.tensor_scalar_mul(w_t[:, 0:HH], v_t[:, 0:HH], cs)
                nc.vector.tensor_scalar_mul(w_t[:, HH:HW_], v_t[:, HH:HW_],
                                            cs)
                # 6. u += w (in-place add), in 3 pieces so the next
                # step's pass-1 ops (whose reads cross the half
                # boundary by one element) are never adjacent to the
                # piece they depend on
                M0, M1 = HH - 512, HH + 512
                nc.vector.tensor_add(u_t[:, 0:M0], u_t[:, 0:M0],
                                     w_t[:, 0:M0])
                nc.vector.tensor_add(u_t[:, M0:M1], u_t[:, M0:M1],
                                     w_t[:, M0:M1])
                nc.vector.tensor_add(u_t[:, M1:HW_], u_t[:, M1:HW_],
                                     w_t[:, M1:HW_])

            def epilogue(u_t, k):
                # avgpool 2x2 * f -> f32 into the dead r tile, DMA out
                u4 = u_t[:, :].rearrange("p (h w2 two) -> p h w2 two",
                                         h=H, w2=W // 2, two=2)
                v3 = t3(v_t)
                hv = H // 2
                for a, b in ((0, hv), (hv, H)):
                    nc.vector.tensor_add(v3[:, a:b, 0:64],
                                         u4[:, a:b, :, 0:1],
                                         u4[:, a:b, :, 1:2])
                a2 = v_t[:, :].rearrange("p (h2 two w) -> p h2 two w",
                                         h2=hv, two=2, w=W)
                for a, b in ((0, hv // 2), (hv // 2, hv)):
                    nc.vector.tensor_add(v3[:, a:b, 64:128],
                                         a2[:, a:b, 0:1, 0:64],
                                         a2[:, a:b, 1:2, 0:64])
                r32 = r_t.bitcast(f32)
                o_v = r32[:, W // 2:W // 2 + hv * 64].rearrange(
                    "p (h w) -> p h w", h=hv, w=64)
                fs = cs_t[:, 2 * k + 1:2 * k + 2]
                for a, b in ((0, hv // 2), (hv // 2, hv)):
                    nc.vector.tensor_scalar_mul(o_v[:, a:b, :],
                                                v3[:, a:b, 64:128], fs)
                rows = slice(k * CHUNK, (k + 1) * CHUNK)
                nc.sync.dma_start(
                    y_d[rows, :].rearrange("p (h w) -> p h w", h=hv, w=64),
                    o_v)

            def body():
                dma_in(0, 0)
                dma_in(0, 1)
                for k in range(NCHUNK):
                    u_t, w_t = u_ts[k % 2], u_ts[(k + 1) % 2]
                    convert(u_t, 0)
                    convert(u_t, 1)
                    if k + 1 < NCHUNK:
                        dma_in(k + 1, 0)
                        dma_in(k + 1, 1)
                    for _ in range(T):
                        step(u_t, w_t, k)
                    epilogue(u_t, k)

            if reps is None:
                body()
            else:
                with tc.For_i(0, reps):
                    for _ in range(bodies):
                        body()

    nc.compile()
    return nc


def build_program(reps=None, bodies=1):
    key = ("nc", reps, bodies)
    if key not in _CACHE:
        _CACHE[key] = _build_program(reps, bodies)
    return _CACHE[key]


def kernel(x: np.ndarray, pool_strength: np.ndarray) -> np.ndarray:
    from concourse.bass_utils import run_bass_kernel_spmd

    nc = build_program()

    x = np.asarray(x, dtype=np.float32)
    s = np.asarray(pool_strength, dtype=np.float64).reshape(C)
    c_ch = (s / (1.0 - s)).astype(np.float32)
    f_ch = (((1.0 - s) ** T) * 0.25).astype(np.float32)
    cvec = np.ascontiguousarray(np.tile(c_ch, B_PER_CORE)[:, None])
    fvec = np.ascontiguousarray(np.tile(f_ch, B_PER_CORE)[:, None])

    in_maps = []
    for j in range(N_CORES):
        xj = np.ascontiguousarray(
            x[j * B_PER_CORE:(j + 1) * B_PER_CORE].reshape(IMGS, HW_))
        in_maps.append({"x": xj, "cvec": cvec, "fvec": fvec})

    res = run_bass_kernel_spmd(nc, in_maps, list(range(N_CORES)))

    out = np.empty((B, C, H // 2, W // 2), dtype=np.float32)
    for j in range(N_CORES):
        yj = res.results[j]["y"].reshape(B_PER_CORE, C, H // 2, W // 2)
        out[j * B_PER_CORE:(j + 1) * B_PER_CORE] = yj
    return out
